# Initial kernel scaffold
#
"""Trainium2 Bass kernel for nn_AgMixPooler (segment_reduce).

Strategy (data-parallel over B across 8 cores, 2 samples/core):
  - Stream X[b] in [128 token, 1024 E] tiles (contiguous DMA).
  - PE-transpose each tile into a per-chunk fp32r strip [128 E, 8 eb, 524]
    (512 tokens + 3-token halos + guard) for the 7-tap conv projection.
  - Conv logits: pair-folded fp32r matmuls (PSUM accumulation performs the
    first level of the band sum across the 7 taps); a small DVE tree
    finishes the shifted sum. ssf projection rides along as extra
    contraction rows using a host-transposed ssf input.
  - a = tanh(alpha*conv + (1-alpha)*ssf + beta); w = exp(a) (the softmax
    normalizer cancels inside each 8-token segment, so pooled needs only
    local exponentials; the tiny attn output is scaled by the global 1/Z
    at sample end).
  - Pooling: per 128-token tile build Wsel[t, s] = w_norm[t] * (t//8 == s)
    where w_norm = w / max(seg_w, eps); one matmul Wsel.T @ X gives the
    pooled [16, 1024] block directly.
"""

import os
import numpy as np

import concourse.bass as bass
import concourse.tile as tile
from concourse import mybir
from concourse.bass_utils import run_bass_kernel_spmd

F32 = mybir.dt.float32
F32R = mybir.dt.float32r
AF = mybir.ActivationFunctionType
ALU = mybir.AluOpType

B, T, E, L, WIN = 16, 4096, 1024, 512, 7
N_CORES = 8
B_PC = B // N_CORES          # samples per core
CH = 512                     # tokens per chunk
NCH = T // CH                # chunks per sample
TPC = CH // 128              # 128-token tiles per chunk
HALF = 256                   # band-sum half width
PW = 260                     # proj matmul moving width (>=256 for f32r rate)
SW = 524                     # strip width: 3 + 512 + 3 + 6 guard
SSF_PAD = 3 + T + 9          # host-padded ssfT width
EPS = 1e-8

# ---------------------------------------------------------------- patches


def _install_compat():
    """walrus CoreV3 codegen in this container accepts at most ONE sem wait
    per non-DMA instruction; the Tile scheduler attaches several. Split the
    extras onto same-engine NoOps inserted before the instruction."""
    import concourse.tile as tile_mod

    if getattr(tile_mod.TileContext, "_waitsplit_patched", False):
        return

    dma_types = tuple(
        getattr(mybir, n)
        for n in ("InstTensorLoad", "InstTensorSave", "InstDMA",
                  "InstDmaTrigger", "InstDmaTransposeAnt", "InstTensorCopyDma")
        if hasattr(mybir, n)
    )
    counter = [0]

    def _split_block_waits(nc, blk, max_waits=1):
        insts = blk.instructions
        out = []
        changed = False
        for inst in insts:
            si = inst.sync_info
            waits = list(si.on_wait) if si and si.on_wait else []
            if len(waits) > max_waits and not isinstance(inst, dma_types):
                for i in range(0, len(waits) - max_waits, max_waits):
                    counter[0] += 1
                    nop = mybir.InstNoOp(
                        name=f"waitsplit-{counter[0]}", ins=[], outs=[])
                    nop.engine = inst.engine
                    nop.sync_info = mybir.SyncInfo(
                        on_wait=waits[i:i + max_waits], on_update=[])
                    nc.register_instruction(nop, overwrite=True)
                    out.append(nop)
                si.on_wait = waits[len(waits) - max_waits:]
                changed = True
            out.append(inst)
        if changed:
            blk.instructions = out

    orig = tile_mod.TileContext.schedule_and_allocate

    def schedule_and_allocate(self, validate_deps=False):
        r = orig(self, validate_deps)
        for f in self.nc.m.functions:
            for blk in f.blocks:
                _split_block_waits(self.nc, blk)
        return r

    tile_mod.TileContext.schedule_and_allocate = schedule_and_allocate
    tile_mod.TileContext._waitsplit_patched = True


def _ap(t, row0, nrows, off, dims):
    """Manual AP on a tile: partition slice [row0, row0+nrows) plus an
    element offset and explicit free dims [[step, count], ...]."""
    pstep = t.ap[0][0]
    return bass.AP(tensor=t.tensor, offset=t.offset + row0 * pstep + off,
                   ap=[[pstep, nrows]] + dims)


# ---------------------------------------------------------------- builder

def build_nc(use_mask=False, pool_f32r=True):
    nc = bass.Bass()

    x = nc.dram_tensor("x", [B_PC, T, E], F32, kind="ExternalInput")
    ssfT = nc.dram_tensor("ssfT", [B_PC, WIN, SSF_PAD], F32,
                          kind="ExternalInput")
    kpk = nc.dram_tensor("kpk", [128, 8, 8], F32, kind="ExternalInput")
    ssfk = nc.dram_tensor("ssfk", [WIN, 4], F32, kind="ExternalInput")
    beta = nc.dram_tensor("beta", [1, 1], F32, kind="ExternalInput")
    m16 = nc.dram_tensor("m16", [128, 16], F32, kind="ExternalInput")
    ident = nc.dram_tensor("ident", [128, 128], F32, kind="ExternalInput")
    if use_mask:
        maskf = nc.dram_tensor("maskf", [B_PC, T], F32, kind="ExternalInput")
    pooled = nc.dram_tensor("pooled", [B_PC, L, E], F32, kind="ExternalOutput")
    attn = nc.dram_tensor("attn", [B_PC, T], F32, kind="ExternalOutput")

    with tile.TileContext(nc) as tc:
        consts = tc.tile_pool(name="consts", bufs=1).__enter__()
        xp = tc.tile_pool(name="xp", bufs=6).__enter__()
        xrp = tc.tile_pool(name="xrp", bufs=10).__enter__()
        stripp = tc.tile_pool(name="stripp", bufs=3).__enter__()
        ssfp = tc.tile_pool(name="ssfp", bufs=2).__enter__()
        smallp = tc.tile_pool(name="smallp", bufs=4).__enter__()
        wexpp = tc.tile_pool(name="wexpp", bufs=2 * NCH + 2).__enter__()
        outp = tc.tile_pool(name="outp", bufs=3).__enter__()
        xtps = tc.tile_pool(name="xtps", bufs=2, space="PSUM").__enter__()
        ppps = tc.tile_pool(name="ppps", bufs=2, space="PSUM").__enter__()
        poolps = tc.tile_pool(name="poolps", bufs=2, space="PSUM").__enter__()

        # ---- constants
        ident_sb = consts.tile([128, 128], F32)
        nc.sync.dma_start(out=ident_sb, in_=ident.ap())
        m16_sb = consts.tile([128, 16], F32)
        nc.sync.dma_start(out=m16_sb, in_=m16.ap())
        kpk_sb = consts.tile([128, 8, 8], F32)
        nc.sync.dma_start(out=kpk_sb, in_=kpk.ap())
        kr = consts.tile([128, 8, 8], F32R)
        nc.vector.tensor_copy(kr, kpk_sb)
        ssfk_sb = consts.tile([WIN, 4], F32)
        nc.sync.dma_start(out=ssfk_sb, in_=ssfk.ap())
        ssfkr = consts.tile([WIN, 4], F32R)
        nc.vector.tensor_copy(ssfkr, ssfk_sb)
        beta_sb = consts.tile([1, 1], F32)
        nc.sync.dma_start(out=beta_sb, in_=beta.ap())
        eps_sb = consts.tile([1, 1], F32)
        nc.vector.memset(eps_sb, EPS)

        pool_dt = F32R if pool_f32r else F32

        def emit_chunk_load(b, c, prev_strip):
            """DMA + transpose chunk c of sample b into a fresh strip.
            Returns (strip, xr_tiles, xt_ps_tiles)."""
            t0 = c * CH
            strip = stripp.tile([128, 8, SW], F32R, tag="strip")
            xts = []
            xrs = []
            for j in range(TPC):
                x_sb = xp.tile([128, E], F32, tag="x")
                nc.sync.dma_start(
                    out=x_sb, in_=x.ap()[b, t0 + 128 * j: t0 + 128 * (j + 1), :])
                if pool_f32r:
                    xr = xrp.tile([128, E], F32R, tag="xr")
                    nc.gpsimd.tensor_copy(xr, x_sb)
                else:
                    xr = x_sb
                xrs.append(xr)
                for g in range(2):  # eb groups 0-3 / 4-7
                    ps = xtps.tile([128, 512], F32, tag="xt")
                    xts.append(ps)
                    for e in range(4):
                        eb = 4 * g + e
                        nc.tensor.transpose(
                            ps[:, 128 * e: 128 * (e + 1)],
                            x_sb[:, 128 * eb: 128 * (eb + 1)], ident_sb)
                    # copy 4 eb blocks into the strip (rounds to f32r)
                    nc.vector.tensor_copy(
                        strip[:, 4 * g: 4 * g + 4, 3 + 128 * j: 3 + 128 * (j + 1)],
                        ps.rearrange("p (a q) -> p a q", a=4))
            # guard columns (avoid NaN garbage reaching the band tree)
            nc.gpsimd.memset(strip[:, :, 518:SW], 0.0)
            if c == 0:
                nc.gpsimd.memset(strip[:, :, 0:3], 0.0)
            else:
                nc.vector.tensor_copy(strip[:, :, 0:3],
                                      prev_strip[:, :, 512:515])
            # ssf strip
            ssft = ssfp.tile([WIN, SW], F32, tag="ssft")
            nc.sync.dma_start(out=ssft, in_=ssfT.ap()[b, :, t0: t0 + SW])
            ssftr = ssfp.tile([WIN, SW], F32R, tag="ssftr")
            nc.gpsimd.tensor_copy(ssftr, ssft)
            return strip, xrs, xts, ssftr

        def emit_right_halo(strip_prev, xt_ps_next):
            # strip cols [515:518) = first 3 tokens of the next chunk
            for g in range(2):
                nc.vector.tensor_copy(
                    strip_prev[:, 4 * g: 4 * g + 4, 515:518],
                    xt_ps_next[g].rearrange("p (a q) -> p a q", a=4)[:, :, 0:3])

        def emit_chunk_compute(b, c, strip, xrs, ssftr, zrow, wexps, mrow):
            t0 = c * CH
            # ---- projection + band sum -> wrow [1, 512]
            wrow = smallp.tile([1, CH], F32, tag="wrow")
            for h in range(2):
                bh = h * HALF
                pp = ppps.tile([4, PW], F32, tag="pp")
                first = True
                for eb in range(8):
                    nc.tensor.matmul(pp, kr[:, eb, 0:4],
                                     strip[:, eb, bh: bh + PW],
                                     start=first, stop=False)
                    first = False
                    nc.tensor.matmul(pp, kr[:, eb, 4:8],
                                     strip[:, eb, bh + 4: bh + 4 + PW],
                                     start=False, stop=False)
                nc.tensor.matmul(pp, ssfkr, ssftr[:, bh: bh + PW],
                                 start=False, stop=True)
                pab = smallp.tile([4, PW], F32, tag="pab")
                nc.scalar.copy(pab, pp)
                flat = smallp.tile([1, 4, PW], F32, tag="flat")
                nc.sync.dma_start(out=flat, in_=pab)
                s2 = smallp.tile([1, 2, PW - 2], F32, tag="s2")
                nc.vector.tensor_add(
                    s2, flat[:, 0:2, 0: PW - 2],
                    _ap(flat, 0, 1, 2 * PW + 2, [[PW, 2], [1, PW - 2]]))
                nc.vector.tensor_add(
                    wrow[:, bh: bh + HALF], s2[:, 0, 0:HALF],
                    _ap(s2, 0, 1, (PW - 2) + 1, [[1, HALF]]))
            # ---- activations
            arow = smallp.tile([1, CH], F32, tag="arow")
            nc.scalar.activation(out=arow, in_=wrow, func=AF.Tanh,
                                 bias=beta_sb, scale=1.0)
            wexp = wexpp.tile([1, CH], F32, tag="wexp")
            nc.scalar.activation(out=wexp, in_=arow, func=AF.Exp)
            if mrow is not None:
                nc.vector.tensor_mul(wexp, wexp, mrow)
            wexps.append(wexp)
            # ---- segment weights
            segw = smallp.tile([1, L // NCH], F32, tag="segw")
            nc.vector.reduce_sum(segw,
                                 wexp.rearrange("p (s j) -> p s j", j=8),
                                 axis=mybir.AxisListType.X)
            nc.vector.tensor_scalar(
                out=segw, in0=segw, scalar1=eps_sb, scalar2=None,
                op0=ALU.max)
            segr = smallp.tile([1, L // NCH], F32, tag="segr")
            nc.vector.reciprocal(segr, segw)
            # chunk Z contribution (sum of segw == sum of wexp)
            nc.vector.reduce_sum(zrow[:, c: c + 1], segw,
                                 axis=mybir.AxisListType.X)
            wnorm = smallp.tile([1, CH], F32, tag="wnorm")
            nc.vector.tensor_mul(
                wnorm.rearrange("p (s j) -> p s j", j=8),
                wexp.rearrange("p (s j) -> p s j", j=8),
                _ap(segr, 0, 1, 0, [[1, L // NCH], [0, 8]]))
            # ---- flip to per-token column layout [128, 4]
            wq = smallp.tile([4, 128], F32, tag="wq")
            nc.sync.dma_start(out=wq, in_=wnorm.rearrange("p (a q) -> (p a) q", a=4))
            wcol = ppps.tile([128, 4], F32, tag="pp")
            nc.tensor.transpose(wcol, wq, ident_sb[0:4, 0:4])
            # ---- pooling per 128-token tile
            for j in range(TPC):
                wsel = smallp.tile([128, 16], pool_dt, tag="wsel")
                nc.scalar.activation(out=wsel, in_=m16_sb, func=AF.Copy,
                                     scale=wcol[:, j: j + 1])
                pps = poolps.tile([16, E], F32, tag="pool")
                nc.tensor.matmul(pps[:, 0:512], wsel, xrs[j][:, 0:512],
                                 start=True, stop=True)
                nc.tensor.matmul(pps[:, 512:1024], wsel, xrs[j][:, 512:1024],
                                 start=True, stop=True)
                pool_sb = outp.tile([16, E], F32, tag="pool_sb")
                nc.scalar.copy(pool_sb, pps)
                s0 = (t0 + 128 * j) // 8
                nc.sync.dma_start(out=pooled.ap()[b, s0: s0 + 16, :],
                                  in_=pool_sb)

        for b in range(B_PC):
            zrow = smallp.tile([1, NCH], F32, tag="zrow")
            wexps = []
            mrows = []
            pend = None  # (strip, xrs, ssftr, c)
            for c in range(NCH):
                mrow = None
                if use_mask:
                    mrow = smallp.tile([1, CH], F32, tag="mrow")
                    nc.sync.dma_start(
                        out=mrow, in_=maskf.ap()[b, c * CH:(c + 1) * CH][None, :])
                mrows.append(mrow)
                strip, xrs, xts, ssftr = emit_chunk_load(
                    b, c, pend[0] if pend else None)
                if pend:
                    emit_right_halo(pend[0], xts[0:2])
                    emit_chunk_compute(b, pend[3], pend[0], pend[1], pend[2],
                                       zrow, wexps, mrows[pend[3]])
                pend = (strip, xrs, ssftr, c)
            nc.gpsimd.memset(pend[0][:, :, 515:518], 0.0)
            emit_chunk_compute(b, pend[3], pend[0], pend[1], pend[2],
                               zrow, wexps, mrows[pend[3]])
            # ---- sample epilogue: attn = wexp / Z
            z = smallp.tile([1, 1], F32, tag="z")
            nc.vector.reduce_sum(z, zrow, axis=mybir.AxisListType.X)
            zr = smallp.tile([1, 1], F32, tag="zr")
            nc.vector.reciprocal(zr, z)
            for c in range(NCH):
                arow = smallp.tile([1, CH], F32, tag="attn_row")
                nc.scalar.activation(out=arow, in_=wexps[c], func=AF.Copy,
                                     scale=zr)
                nc.sync.dma_start(
                    out=attn.ap()[b, c * CH:(c + 1) * CH][None, :], in_=arow)

        for p in (consts, xp, xrp, stripp, ssfp, smallp, wexpp, outp,
                  xtps, ppps, poolps):
            p.__exit__(None, None, None)

    return nc


_CACHE = {}


def _get_nc(use_mask, pool_f32r=True):
    key = (use_mask, pool_f32r)
    if key not in _CACHE:
        _install_compat()
        _CACHE[key] = build_nc(use_mask=use_mask, pool_f32r=pool_f32r)
    return _CACHE[key]


def kernel(l_full_embs, ssf_x, padding_mask, conv_w, conv_b, ssf_weight,
           ssf_bias, gate_logit):
    l_full_embs = np.asarray(l_full_embs, dtype=np.float32)
    ssf_x = np.asarray(ssf_x, dtype=np.float32)
    padding_mask = np.asarray(padding_mask)
    conv_w = np.asarray(conv_w, dtype=np.float32)
    conv_b = np.asarray(conv_b, dtype=np.float32)
    ssf_weight = np.asarray(ssf_weight, dtype=np.float32)
    ssf_bias = np.asarray(ssf_bias, dtype=np.float32)
    gate_logit = np.asarray(gate_logit, dtype=np.float32)

    use_mask = not bool(padding_mask.all())
    nc = _get_nc(use_mask)

    # ---- tiny host-side parameter packing
    alpha = 1.0 / (1.0 + np.exp(-float(gate_logit.reshape(-1)[0])))
    beta = np.array([[alpha * float(conv_b.reshape(-1)[0])
                      + (1.0 - alpha) * float(ssf_bias.reshape(-1)[0])]],
                    dtype=np.float32)
    # kpk[p, eb, c] = alpha * conv_w[0, 0, c, eb*128 + p], c<7; 0 for c=7
    kw = (alpha * conv_w[0, 0]).astype(np.float32)          # [7, 1024]
    kpk = np.zeros((128, 8, 8), dtype=np.float32)
    kpk[:, :, 0:7] = kw.reshape(7, 8, 128).transpose(2, 1, 0)
    ssfk = np.zeros((WIN, 4), dtype=np.float32)
    ssfk[:, 3] = (1.0 - alpha) * ssf_weight
    m16v = (np.arange(128)[:, None] // 8 == np.arange(16)[None, :]
            ).astype(np.float32)
    identv = np.eye(128, dtype=np.float32)
    # ssfT: [B, 7, pad + T + pad] host transpose of the tiny ssf input
    ssfTv = np.zeros((B, WIN, SSF_PAD), dtype=np.float32)
    ssfTv[:, :, 3: 3 + T] = ssf_x.transpose(0, 2, 1)
    if use_mask:
        maskfv = padding_mask.astype(np.float32)

    in_maps = []
    for core in range(N_CORES):
        b0 = core * B_PC
        m = {
            "x": np.ascontiguousarray(l_full_embs[b0: b0 + B_PC]),
            "ssfT": np.ascontiguousarray(ssfTv[b0: b0 + B_PC]),
            "kpk": kpk, "ssfk": ssfk, "beta": beta,
            "m16": m16v, "ident": identv,
        }
        if use_mask:
            m["maskf"] = np.ascontiguousarray(maskfv[b0: b0 + B_PC])
        in_maps.append(m)

    res = run_bass_kernel_spmd(nc, in_maps, core_ids=list(range(N_CORES)))

    pooled = np.concatenate([res.results[c]["pooled"] for c in range(N_CORES)],
                            axis=0)
    attn = np.concatenate([res.results[c]["attn"] for c in range(N_CORES)],
                          axis=0)[..., None]
    return pooled, attn


if __name__ == "__main__":
    rng = np.random.default_rng(0)
    inputs = {
        "l_full_embs": rng.standard_normal((B, T, E), dtype=np.float32),
        "ssf_x": rng.standard_normal((B, T, 7), dtype=np.float32),
        "padding_mask": np.ones((B, T), dtype=bool),
        "conv_w": (rng.standard_normal((1, 1, 7, E)) / np.sqrt(7 * E)
                   ).astype(np.float32),
        "conv_b": np.zeros(1, np.float32),
        "ssf_weight": rng.standard_normal(7).astype(np.float32),
        "ssf_bias": np.zeros(1, np.float32),
        "gate_logit": np.zeros(1, np.float32),
    }
    p, a = kernel(**inputs)
    print("pooled", p.shape, "attn", a.shape)


# revision 13
# speedup vs baseline: 1.0400x; 1.0400x over previous
"""Trainium2 Bass kernel for nn_AgMixPooler (segment_reduce).

Strategy (data-parallel over B across 8 cores, 2 samples/core):
  - Stream X[b] in [128 token, 1024 E] tiles (contiguous DMA).
  - PE-transpose each tile into a per-chunk fp32r strip [128 E, 8 eb, 524]
    (512 tokens + 3-token halos + guard) for the 7-tap conv projection.
  - Conv logits: pair-folded fp32r matmuls (PSUM accumulation performs the
    first level of the band sum across the 7 taps); a small DVE tree
    finishes the shifted sum. ssf projection rides along as extra
    contraction rows using a host-transposed ssf input.
  - a = tanh(alpha*conv + (1-alpha)*ssf + beta); w = exp(a) (the softmax
    normalizer cancels inside each 8-token segment, so pooled needs only
    local exponentials; the tiny attn output is scaled by the global 1/Z
    at sample end).
  - Pooling: per 128-token tile build Wsel[t, s] = w_norm[t] * (t//8 == s)
    where w_norm = w / max(seg_w, eps); one matmul Wsel.T @ X gives the
    pooled [16, 1024] block directly.
"""

import os
import numpy as np

import concourse.bass as bass
import concourse.tile as tile
from concourse import mybir
from concourse.bass_utils import run_bass_kernel_spmd

F32 = mybir.dt.float32
F32R = mybir.dt.float32r
AF = mybir.ActivationFunctionType
ALU = mybir.AluOpType

B, T, E, L, WIN = 16, 4096, 1024, 512, 7
N_CORES = 8
B_PC = B // N_CORES          # samples per core
CH = 512                     # tokens per chunk
NCH = T // CH                # chunks per sample
TPC = CH // 128              # 128-token tiles per chunk
HALF = 256                   # band-sum half width
PW = 260                     # proj matmul moving width (>=256 for f32r rate)
SW = 524                     # strip width: 3 + 512 + 3 + 6 guard
SSF_PAD = 3 + T + 9          # host-padded ssfT width
EPS = 1e-8

# ---------------------------------------------------------------- patches


def _install_compat():
    """walrus CoreV3 codegen in this container accepts at most ONE sem wait
    per non-DMA instruction; the Tile scheduler attaches several. Split the
    extras onto same-engine NoOps inserted before the instruction."""
    import concourse.tile as tile_mod

    if getattr(tile_mod.TileContext, "_waitsplit_patched", False):
        return

    counter = [0]

    def _split_block_waits(nc, blk, max_waits=1):
        insts = blk.instructions
        out = []
        changed = False
        for inst in insts:
            si = inst.sync_info
            waits = list(si.on_wait) if si and si.on_wait else []
            if len(waits) > max_waits:
                for i in range(0, len(waits) - max_waits, max_waits):
                    counter[0] += 1
                    nop = mybir.InstNoOp(
                        name=f"waitsplit-{counter[0]}", ins=[], outs=[])
                    nop.engine = inst.engine
                    nop.sync_info = mybir.SyncInfo(
                        on_wait=waits[i:i + max_waits], on_update=[])
                    nc.register_instruction(nop, overwrite=True)
                    out.append(nop)
                si.on_wait = waits[len(waits) - max_waits:]
                changed = True
            out.append(inst)
        if changed:
            blk.instructions = out

    orig = tile_mod.TileContext.schedule_and_allocate

    def schedule_and_allocate(self, validate_deps=False):
        r = orig(self, validate_deps)
        for f in self.nc.m.functions:
            for blk in f.blocks:
                _split_block_waits(self.nc, blk)
        return r

    tile_mod.TileContext.schedule_and_allocate = schedule_and_allocate
    tile_mod.TileContext._waitsplit_patched = True


def _ap(t, row0, nrows, off, dims):
    """Manual AP on a tile: partition slice [row0, row0+nrows) plus an
    element offset and explicit free dims [[step, count], ...]."""
    pstep = t.ap[0][0]
    return bass.AP(tensor=t.tensor, offset=t.offset + row0 * pstep + off,
                   ap=[[pstep, nrows]] + dims)


# ---------------------------------------------------------------- builder

def build_nc(use_mask=False, pool_f32r=True):
    nc = bass.Bass()

    x = nc.dram_tensor("x", [B_PC, T, E], F32, kind="ExternalInput")
    ssfT = nc.dram_tensor("ssfT", [B_PC, WIN, SSF_PAD], F32,
                          kind="ExternalInput")
    kpk = nc.dram_tensor("kpk", [128, 8, 8], F32, kind="ExternalInput")
    ssfk = nc.dram_tensor("ssfk", [WIN, 4], F32, kind="ExternalInput")
    beta = nc.dram_tensor("beta", [1, 1], F32, kind="ExternalInput")
    m16 = nc.dram_tensor("m16", [128, 16], F32, kind="ExternalInput")
    ident = nc.dram_tensor("ident", [128, 128], F32, kind="ExternalInput")
    if use_mask:
        maskf = nc.dram_tensor("maskf", [B_PC, T], F32, kind="ExternalInput")
    pooled = nc.dram_tensor("pooled", [B_PC, L, E], F32, kind="ExternalOutput")
    attn = nc.dram_tensor("attn", [B_PC, T], F32, kind="ExternalOutput")

    from contextlib import ExitStack
    with tile.TileContext(nc) as tc, ExitStack() as es:
        consts = es.enter_context(tc.tile_pool(name="consts", bufs=1))
        xp = es.enter_context(tc.tile_pool(name="xp", bufs=2))
        xrp = es.enter_context(tc.tile_pool(name="xrp", bufs=2))
        stripp = es.enter_context(tc.tile_pool(name="stripp", bufs=2))
        ssfp = es.enter_context(tc.tile_pool(name="ssfp", bufs=2))
        smallp = es.enter_context(tc.tile_pool(name="smallp", bufs=2))
        wexpp = es.enter_context(tc.tile_pool(name="wexpp", bufs=2))
        outp = es.enter_context(tc.tile_pool(name="outp", bufs=2))
        xtps = es.enter_context(tc.tile_pool(name="xtps", bufs=2, space="PSUM"))
        ppps = es.enter_context(tc.tile_pool(name="ppps", bufs=2, space="PSUM"))
        poolps = es.enter_context(tc.tile_pool(name="poolps", bufs=2, space="PSUM"))

        # ---- constants
        ident_sb = consts.tile([128, 128], F32)
        nc.sync.dma_start(out=ident_sb, in_=ident.ap())
        m16_sb = consts.tile([128, 16], F32)
        nc.sync.dma_start(out=m16_sb, in_=m16.ap())
        kpk_sb = consts.tile([128, 8, 8], F32)
        nc.sync.dma_start(out=kpk_sb, in_=kpk.ap())
        kr = consts.tile([128, 8, 8], F32R)
        nc.vector.tensor_copy(kr, kpk_sb)
        ssfk_sb = consts.tile([WIN, 4], F32)
        nc.sync.dma_start(out=ssfk_sb, in_=ssfk.ap())
        ssfkr = consts.tile([WIN, 4], F32R)
        nc.vector.tensor_copy(ssfkr, ssfk_sb)
        beta_sb = consts.tile([1, 1], F32)
        nc.sync.dma_start(out=beta_sb, in_=beta.ap())
        eps_sb = consts.tile([1, 1], F32)
        nc.vector.memset(eps_sb, EPS)


        pool_dt = F32R if pool_f32r else F32

        def emit_chunk_load(b, c, prev_strip):
            """DMA + transpose chunk c of sample b into a fresh strip.
            Returns (strip, xr_tiles, xt_ps_tiles)."""
            t0 = c * CH
            strip = stripp.tile([128, 8, SW], F32R, tag="strip")
            xts = []
            x_sb = xp.tile([128, TPC, E], F32, tag="x")
            nc.sync.dma_start(
                out=x_sb,
                in_=x.ap()[b, t0: t0 + CH, :].rearrange("(j p) e -> p j e", p=128))
            if pool_f32r:
                xr_ch = xrp.tile([128, TPC, E], F32R, tag="xr")
                nc.gpsimd.tensor_copy(xr_ch, x_sb)
                xrs = [xr_ch[:, j, :] for j in range(TPC)]
                tsrc, tid = x_sb, ident_sb
            else:
                xrs = [x_sb[:, j, :] for j in range(TPC)]
                tsrc, tid = x_sb, ident_sb
            for j in range(TPC):
                for g in range(2):  # eb groups 0-3 / 4-7
                    ps = xtps.tile([128, 512], F32, tag="xt")
                    xts.append(ps)
                    for e in range(4):
                        eb = 4 * g + e
                        nc.tensor.transpose(
                            ps[:, 128 * e: 128 * (e + 1)],
                            tsrc[:, j, 128 * eb: 128 * (eb + 1)], tid)
                    # copy 4 eb blocks into the strip (rounds to f32r)
                    nc.vector.tensor_copy(
                        strip[:, 4 * g: 4 * g + 4, 3 + 128 * j: 3 + 128 * (j + 1)],
                        ps.rearrange("p (a q) -> p a q", a=4))
            # guard columns (avoid NaN garbage reaching the band tree)
            nc.gpsimd.memset(strip[:, :, 518:SW].bitcast(F32), 0.0)
            if c == 0:
                nc.gpsimd.memset(strip[:, :, 0:3].bitcast(F32), 0.0)
            else:
                nc.vector.tensor_copy(strip[:, :, 0:3],
                                      prev_strip[:, :, 512:515])
            # ssf strip
            ssft = ssfp.tile([WIN, SW], F32, tag="ssft")
            nc.scalar.dma_start(out=ssft, in_=ssfT.ap()[b, :, t0: t0 + SW])
            ssftr = ssfp.tile([WIN, SW], F32R, tag="ssftr")
            nc.gpsimd.tensor_copy(ssftr, ssft)
            return strip, xrs, xts, ssftr

        def emit_right_halo(strip_prev, xt_ps_next):
            # strip cols [515:518) = first 3 tokens of the next chunk
            for g in range(2):
                nc.vector.tensor_copy(
                    strip_prev[:, 4 * g: 4 * g + 4, 515:518],
                    xt_ps_next[g].rearrange("p (a q) -> p a q", a=4)[:, :, 0:3])

        def emit_chunk_compute(b, c, strip, xrs, ssftr, zrow, wexps, mrow):
            t0 = c * CH
            # ---- projection + band sum -> wrow [1, 512]
            wrow = smallp.tile([1, CH], F32, tag="wrow", bufs=3)
            pab = smallp.tile([4, 2, PW], F32, tag="pab", bufs=3)
            for h in range(2):
                bh = h * HALF
                pp = ppps.tile([4, PW], F32, tag="pp")
                first = True
                for eb in range(8):
                    nc.tensor.matmul(pp, kr[:, eb, 0:4],
                                     strip[:, eb, bh: bh + PW],
                                     start=first, stop=False)
                    first = False
                    nc.tensor.matmul(pp, kr[:, eb, 4:8],
                                     strip[:, eb, bh + 4: bh + 4 + PW],
                                     start=False, stop=False)
                nc.tensor.matmul(pp, ssfkr, ssftr[:, bh: bh + PW],
                                 start=False, stop=True)
                nc.scalar.copy(pab[:, h, :], pp)
            flat = smallp.tile([1, 4, 2, PW], F32, tag="flat", bufs=3)
            nc.sync.dma_start(out=flat, in_=pab)
            for h in range(2):
                bh = h * HALF
                s2 = smallp.tile([1, 2, PW - 2], F32, tag="s2")
                nc.vector.tensor_add(
                    s2,
                    _ap(flat, 0, 1, h * PW, [[2 * PW, 2], [1, PW - 2]]),
                    _ap(flat, 0, 1, (4 + h) * PW + 2, [[2 * PW, 2], [1, PW - 2]]))
                nc.vector.tensor_add(
                    wrow[:, bh: bh + HALF], s2[:, 0, 0:HALF],
                    _ap(s2, 0, 1, (PW - 2) + 1, [[1, HALF]]))
            # ---- activations
            arow = smallp.tile([1, CH], F32, tag="arow")
            nc.scalar.activation(out=arow, in_=wrow, func=AF.Tanh,
                                 bias=beta_sb, scale=1.0)
            wexp = wexps[0][:, c * CH:(c + 1) * CH]
            nc.scalar.activation(out=wexp, in_=arow, func=AF.Exp)
            if mrow is not None:
                nc.vector.tensor_mul(wexp, wexp, mrow)
            # ---- segment weights
            segw = smallp.tile([1, L // NCH], F32, tag="segw")
            nc.vector.reduce_sum(segw,
                                 wexp.rearrange("p (s j) -> p s j", j=8),
                                 axis=mybir.AxisListType.X)
            nc.vector.tensor_scalar(
                out=segw, in0=segw, scalar1=eps_sb, scalar2=None,
                op0=ALU.max)
            segr = smallp.tile([1, L // NCH], F32, tag="segr")
            nc.vector.reciprocal(segr, segw)
            # chunk Z contribution (sum of segw == sum of wexp)
            nc.vector.reduce_sum(zrow[:, c: c + 1], segw,
                                 axis=mybir.AxisListType.X)
            wnorm = smallp.tile([1, CH], F32, tag="wnorm")
            nc.vector.tensor_mul(
                wnorm.rearrange("p (s j) -> p s j", j=8),
                wexp.rearrange("p (s j) -> p s j", j=8),
                _ap(segr, 0, 1, 0, [[1, L // NCH], [0, 8]]))
            # ---- flip to per-token column layout [128, 4]
            wq = smallp.tile([4, 128], F32, tag="wq")
            nc.scalar.dma_start(out=wq, in_=wnorm.rearrange("p (a q) -> (p a) q", a=4))
            wcol_ps = ppps.tile([128, 4], F32, tag="pp")
            nc.tensor.transpose(wcol_ps, wq, ident_sb[0:4, 0:4])
            wcol = smallp.tile([128, 4], F32, tag="wcol")
            nc.vector.tensor_copy(wcol, wcol_ps)
            # ---- pooling per 128-token tile
            for j in range(TPC):
                wsel = smallp.tile([128, 16], pool_dt, tag="wsel")
                nc.vector.tensor_scalar_mul(wsel, m16_sb, wcol[:, j: j + 1])
                pps = poolps.tile([16, E], F32, tag="pool")
                nc.tensor.matmul(pps[:, 0:512], wsel, xrs[j][:, 0:512],
                                 start=True, stop=True)
                nc.tensor.matmul(pps[:, 512:1024], wsel, xrs[j][:, 512:1024],
                                 start=True, stop=True)
                pool_sb = outp.tile([16, E], F32, tag="pool_sb")
                nc.scalar.copy(pool_sb, pps)
                s0 = (t0 + 128 * j) // 8
                nc.gpsimd.dma_start(out=pooled.ap()[b, s0: s0 + 16, :],
                                     in_=pool_sb)

        for b in range(B_PC):
            zrow = smallp.tile([1, NCH], F32, tag="zrow")
            wexp_t = wexpp.tile([1, T], F32, tag="wexp")
            wexps = [wexp_t]
            mrows = []
            pend = None  # (strip, xrs, ssftr, c)
            for c in range(NCH):
                mrow = None
                if use_mask:
                    mrow = smallp.tile([1, CH], F32, tag="mrow")
                    nc.sync.dma_start(
                        out=mrow, in_=maskf.ap()[b, c * CH:(c + 1) * CH][None, :])
                mrows.append(mrow)
                strip, xrs, xts, ssftr = emit_chunk_load(
                    b, c, pend[0] if pend else None)
                if pend:
                    emit_right_halo(pend[0], xts[0:2])
                    emit_chunk_compute(b, pend[3], pend[0], pend[1], pend[2],
                                       zrow, wexps, mrows[pend[3]])
                pend = (strip, xrs, ssftr, c)
            nc.gpsimd.memset(pend[0][:, :, 515:518].bitcast(F32), 0.0)
            emit_chunk_compute(b, pend[3], pend[0], pend[1], pend[2],
                               zrow, wexps, mrows[pend[3]])
            # ---- sample epilogue: attn = wexp / Z
            z = smallp.tile([1, 1], F32, tag="z")
            nc.vector.reduce_sum(z, zrow, axis=mybir.AxisListType.X)
            zr = smallp.tile([1, 1], F32, tag="zr")
            nc.vector.reciprocal(zr, z)
            for c in range(NCH):
                wexp = wexps[0][:, c * CH:(c + 1) * CH]
                arow = smallp.tile([1, CH], F32, tag="attn_row")
                nc.vector.tensor_scalar_mul(arow, wexp, zr)
                nc.gpsimd.dma_start(
                    out=attn.ap()[b, c * CH:(c + 1) * CH][None, :], in_=arow)

    return nc


_CACHE = {}


def _get_nc(use_mask, pool_f32r=True):
    key = (use_mask, pool_f32r)
    if key not in _CACHE:
        _install_compat()
        _CACHE[key] = build_nc(use_mask=use_mask, pool_f32r=pool_f32r)
    return _CACHE[key]


def kernel(l_full_embs, ssf_x, padding_mask, conv_w, conv_b, ssf_weight,
           ssf_bias, gate_logit):
    l_full_embs = np.asarray(l_full_embs, dtype=np.float32)
    ssf_x = np.asarray(ssf_x, dtype=np.float32)
    padding_mask = np.asarray(padding_mask)
    conv_w = np.asarray(conv_w, dtype=np.float32)
    conv_b = np.asarray(conv_b, dtype=np.float32)
    ssf_weight = np.asarray(ssf_weight, dtype=np.float32)
    ssf_bias = np.asarray(ssf_bias, dtype=np.float32)
    gate_logit = np.asarray(gate_logit, dtype=np.float32)

    use_mask = not bool(padding_mask.all())
    nc = _get_nc(use_mask)

    # ---- tiny host-side parameter packing
    alpha = 1.0 / (1.0 + np.exp(-float(gate_logit.reshape(-1)[0])))
    beta = np.array([[alpha * float(conv_b.reshape(-1)[0])
                      + (1.0 - alpha) * float(ssf_bias.reshape(-1)[0])]],
                    dtype=np.float32)
    # kpk[p, eb, c] = alpha * conv_w[0, 0, c, eb*128 + p], c<7; 0 for c=7
    kw = (alpha * conv_w[0, 0]).astype(np.float32)          # [7, 1024]
    kpk = np.zeros((128, 8, 8), dtype=np.float32)
    kpk[:, :, 0:7] = kw.reshape(7, 8, 128).transpose(2, 1, 0)
    ssfk = np.zeros((WIN, 4), dtype=np.float32)
    ssfk[:, 3] = (1.0 - alpha) * ssf_weight
    m16v = (np.arange(128)[:, None] // 8 == np.arange(16)[None, :]
            ).astype(np.float32)
    identv = np.eye(128, dtype=np.float32)
    # ssfT: [B, 7, pad + T + pad] host transpose of the tiny ssf input
    ssfTv = np.zeros((B, WIN, SSF_PAD), dtype=np.float32)
    ssfTv[:, :, 3: 3 + T] = ssf_x.transpose(0, 2, 1)
    if use_mask:
        maskfv = padding_mask.astype(np.float32)

    in_maps = []
    for core in range(N_CORES):
        b0 = core * B_PC
        m = {
            "x": np.ascontiguousarray(l_full_embs[b0: b0 + B_PC]),
            "ssfT": np.ascontiguousarray(ssfTv[b0: b0 + B_PC]),
            "kpk": kpk, "ssfk": ssfk, "beta": beta,
            "m16": m16v, "ident": identv,
        }
        if use_mask:
            m["maskf"] = np.ascontiguousarray(maskfv[b0: b0 + B_PC])
        in_maps.append(m)

    res = run_bass_kernel_spmd(nc, in_maps, core_ids=list(range(N_CORES)))

    pooled = np.concatenate([res.results[c]["pooled"] for c in range(N_CORES)],
                            axis=0)
    attn = np.concatenate([res.results[c]["attn"] for c in range(N_CORES)],
                          axis=0)[..., None]
    return pooled, attn


if __name__ == "__main__":
    rng = np.random.default_rng(0)
    inputs = {
        "l_full_embs": rng.standard_normal((B, T, E), dtype=np.float32),
        "ssf_x": rng.standard_normal((B, T, 7), dtype=np.float32),
        "padding_mask": np.ones((B, T), dtype=bool),
        "conv_w": (rng.standard_normal((1, 1, 7, E)) / np.sqrt(7 * E)
                   ).astype(np.float32),
        "conv_b": np.zeros(1, np.float32),
        "ssf_weight": rng.standard_normal(7).astype(np.float32),
        "ssf_bias": np.zeros(1, np.float32),
        "gate_logit": np.zeros(1, np.float32),
    }
    p, a = kernel(**inputs)
    print("pooled", p.shape, "attn", a.shape)
